# revision 28
# baseline (speedup 1.0000x reference)
"""Trainium2 Bass kernel for a group-conv / orbit-shared message-passing layer.

Math: out[b, i, o] = sum_{j,c} weight[o, c, pair_orbit[i, j]] * x[b, j, c] + bias[o]

Strategy (pure data parallel over 8 NeuronCores):
  * Host gathers the orbit-shared weight into per-output-position matrices
    W_i[(j,c), o] (24 matrices of 1536x64), regrouped as moving operands
    Wmov[k, g][kc, (di,o)] of [128, 512] covering 8 output positions each.
  * Host pre-tiles x so each batch tile is one fully-contiguous DMA:
    xtile[bt][kc, (k,b)] with the contraction dim (j,c)=1536 split into
    12 k-chunks of 128 on SBUF partitions; each core takes B/8 = 4096 rows.
  * Per 128-batch tile: stationary = x k-tile [kc=128, b=128], moving =
    Wmov[k, g] [kc=128, 512]; 12 k-tiles accumulate into 3 PSUM banks:
        psum_g[b, (di,o)] += x[kc, b].T @ Wmov[k,g][kc, (di,o)]
    The PSUM free axis (di,o) is already the natural out[b, i, o] layout, so
    stores go straight to a (4096, 24*64) DRAM tensor.
  * bf16 operands: full PE rate, FWL fast weight loads, half the HBM traffic
    of fp32. L2 rel err ~2.3e-3, well under the 2e-2 gate.
  * Startup: PE pre-warm dummy matmuls (p-state/HAM ramp), weight plane on
    the gpsimd DMA queue, and the first two batch tiles run k-interleaved so
    chunk consumption (6 matmuls/chunk) never outraces chunk delivery.
"""

import os
import sys

for _p in ("/opt/trn_rl_repo",):
    if _p not in sys.path:
        sys.path.insert(0, _p)

import numpy as np
import ml_dtypes

import concourse.bacc as bacc
import concourse.mybir as mybir
from concourse import tile
from concourse.bass_utils import run_bass_kernel_spmd

B, P, C_IN, C_OUT, N_ORB = 32768, 24, 64, 64, 24
N_CORES = 8
BL = B // N_CORES            # 4096 batch per core
JC = P * C_IN                # 1536 contraction size
KT = JC // 128               # 12 K-tiles
NG = 3                       # output groups of 8 positions (8*64 = 512 free)
NBT = BL // 128              # 32 batch tiles per core

# "bf16" | "f32r" | "f32"
COMPUTE_DTYPE = os.environ.get("COMPUTE_DTYPE", "bf16")
N_WARM = int(os.environ.get("N_WARM", "145"))

_CACHE = {}


def _dt(dt_tag):
    return {
        "bf16": mybir.dt.bfloat16,
        "f32r": mybir.dt.float32r,
        "f32": mybir.dt.float32,
    }[dt_tag]


def _build(dt_tag):
    DT = _dt(dt_tag)

    nc = bacc.Bacc(None, target_bir_lowering=False, debug=False)
    # pre-tiled x: row bt*128+kc, col k*128+b (one contiguous DMA per tile)
    xt = nc.dram_tensor("xt", [NBT * 128, KT * 128], DT, kind="ExternalInput")
    w = nc.dram_tensor("w", [128, KT * NG * 512], DT, kind="ExternalInput")
    out_l = nc.dram_tensor("out_l", [BL, P * C_OUT], mybir.dt.float32,
                           kind="ExternalOutput")

    with tile.TileContext(nc) as tc:
        with (
            tc.tile_pool(name="wpool", bufs=1) as wpool,
            tc.tile_pool(name="xpool", bufs=3) as xpool,
            tc.tile_pool(name="opool", bufs=6) as opool,
            tc.tile_pool(name="pspool", bufs=2, space="PSUM") as pspool,
        ):
            # weight chunks ride the dedicated gpsimd DMA queue, except w2:
            # its DMA is issued on the sync queue (below, after the x0/x1
            # split slices) because the gpsimd queue's early ramp can't
            # deliver three chunks in time while sync sits idle after x0/x1.
            # All 12 tiles are allocated here, in order, so SBUF layout is
            # independent of the queue assignment.
            wk = []
            for k in range(KT):
                wc = wpool.tile([128, NG * 512], DT, tag=f"w{k}", name=f"wc{k}")
                if k != 2:
                    nc.gpsimd.dma_start(
                        wc[:], w.ap()[:, k * NG * 512:(k + 1) * NG * 512]
                    )
                wk.append(wc)

            def load_x(bt):
                xbt = xpool.tile([128, KT * 128], DT, tag="xbt", name=f"xb{bt}")
                nc.sync.dma_start(
                    xbt[:], xt.ap()[bt * 128:(bt + 1) * 128, :]
                )
                return xbt

            # x0/x1 split so their first k-slices land before the full tiles
            # (the PE hoists each LDWEIGHTS one matmul early, so tile bt+0's
            # last k0 matmul would otherwise block on ALL of x1)
            def load_x_split(bt):
                xbt = xpool.tile([128, KT * 128], DT, tag="xbt",
                                 name=f"xb{bt}")
                r = slice(bt * 128, (bt + 1) * 128)
                nc.sync.dma_start(xbt[:, 0:256], xt.ap()[r, 0:256])
                nc.sync.dma_start(xbt[:, 256:KT * 128],
                                  xt.ap()[r, 256:KT * 128])
                return xbt

            x0 = load_x_split(0)
            x1 = load_x_split(1)
            nc.sync.dma_start(
                wk[2][:], w.ap()[:, 2 * NG * 512:3 * NG * 512]
            )

            # PE pre-warm: dummy matmuls keep the PE busy from ~7us until the
            # first weight chunk lands (~13.5us) so the p-state/HAM ramp is
            # complete before the first real matmul issues (~44ns each)
            warm = xpool.tile([128, 64], DT, tag="warm", name="warm")
            nc.vector.memset(warm[:], 0.0)
            pswarm = pspool.tile([64, 64], mybir.dt.float32, tag="pswarm",
                                 name="pswarm")
            for _ in range(N_WARM):
                nc.tensor.matmul(pswarm[:], warm[:], warm[:],
                                 start=True, stop=True)

            def psum_tiles(bt):
                return [
                    pspool.tile([128, 512], mybir.dt.float32, tag=f"ps{g}",
                                name=f"ps{bt}_{g}")
                    for g in range(NG)
                ]

            def mm_k(ps, xtile, k):
                lhsT = xtile[:, k * 128:(k + 1) * 128]
                for g in range(NG):
                    nc.tensor.matmul(
                        ps[g][:],
                        lhsT,
                        wk[k][:, g * 512:(g + 1) * 512],
                        start=(k == 0),
                        stop=(k == KT - 1),
                    )

            def drain(bt, ps):
                for g in range(NG):
                    ob = opool.tile([128, 512], mybir.dt.float32, tag="ob",
                                    name=f"ob{bt}_{g}")
                    # drains split across DVE and ACT so consecutive drains
                    # overlap instead of serializing on one engine
                    if g == 1:
                        nc.scalar.copy(ob[:], ps[g][:])
                    else:
                        nc.vector.tensor_copy(ob[:], ps[g][:])
                    nc.sync.dma_start(
                        out_l.ap()[bt * 128:(bt + 1) * 128,
                                   g * 512:(g + 1) * 512],
                        ob[:],
                    )

            # first two batch tiles k-interleaved (see module docstring)
            ps0, ps1 = psum_tiles(0), psum_tiles(1)
            for k in range(KT):
                mm_k(ps0, x0, k)
                mm_k(ps1, x1, k)
            # x2/x3 ride the gpsimd queue BEHIND the weight chunks: they are
            # not needed until ~29/37us, and keeping their bytes out of the
            # early window lets w2 (the critical-path chunk) arrive sooner
            # on the shared early HBM bandwidth
            def load_x_late(bt):
                xbt = xpool.tile([128, KT * 128], DT, tag="xbt",
                                 name=f"xb{bt}")
                nc.gpsimd.dma_start(
                    xbt[:], xt.ap()[bt * 128:(bt + 1) * 128, :]
                )
                return xbt

            xtiles = {2: load_x_late(2)}
            drain(0, ps0)
            xtiles[3] = load_x_late(3)
            drain(1, ps1)

            for bt in range(2, NBT):
                xcur = xtiles.pop(bt)
                ps = psum_tiles(bt)
                for k in range(KT):
                    mm_k(ps, xcur, k)
                if bt + 2 < NBT:
                    xtiles[bt + 2] = load_x(bt + 2)
                drain(bt, ps)

    nc.compile()
    return nc


def _get_nc(dt_tag):
    if dt_tag not in _CACHE:
        _CACHE[dt_tag] = _build(dt_tag)
    return _CACHE[dt_tag]


def _pack_weight(weight, pair_orbit, dt_tag):
    # W_i[(j,c), o] = weight[o, c, pair_orbit[i, j]]
    kern = weight[:, :, np.asarray(pair_orbit)]          # (o, c, i, j)
    wfull = kern.transpose(2, 3, 1, 0).reshape(P, JC, C_OUT)   # (i, jc, o)
    # Wmov[k, g, kc, di*64+o] = wfull[g*8+di, k*128+kc, o]
    wmov = (
        wfull.reshape(NG, 8, KT, 128, C_OUT)
        .transpose(2, 0, 3, 1, 4)
        .reshape(KT * NG, 128, 512)
    )
    wsb = np.ascontiguousarray(
        wmov.transpose(1, 0, 2).reshape(128, KT * NG * 512), dtype=np.float32
    )
    if dt_tag == "bf16":
        return wsb.astype(ml_dtypes.bfloat16)
    return wsb


def _shard_x(x, dt_tag):
    # xtile[bt*128+kc, k*128+b] = x[core*BL + bt*128 + b, k*128 + kc]
    x2 = x.reshape(B, JC)
    if dt_tag == "bf16":
        x2 = x2.astype(ml_dtypes.bfloat16)
    shards = []
    for c in range(N_CORES):
        xc = x2[c * BL:(c + 1) * BL]
        xtile = (
            xc.reshape(NBT, 128, KT, 128)       # (bt, b, k, kc)
            .transpose(0, 3, 2, 1)              # (bt, kc, k, b)
            .reshape(NBT * 128, KT * 128)
        )
        shards.append(np.ascontiguousarray(xtile))
    return shards


def _prepare_inputs(x, weight, pair_orbit, dt_tag):
    wsb = _pack_weight(weight, pair_orbit, dt_tag)
    xts = _shard_x(x, dt_tag)
    return [{"xt": xts[c], "w": wsb} for c in range(N_CORES)]


def kernel(x, weight, bias, pair_orbit):
    x = np.asarray(x, dtype=np.float32)
    weight = np.asarray(weight, dtype=np.float32)
    bias = np.asarray(bias, dtype=np.float32)

    dt_tag = COMPUTE_DTYPE
    nc = _get_nc(dt_tag)
    in_maps = _prepare_inputs(x, weight, pair_orbit, dt_tag)

    res = run_bass_kernel_spmd(nc, in_maps, core_ids=list(range(N_CORES)))

    out = np.concatenate(
        [res.results[c]["out_l"] for c in range(N_CORES)], axis=0
    ).reshape(B, P, C_OUT)
    if bias.any():
        out = out + bias
    return out


# revision 30
# speedup vs baseline: 1.2308x; 1.2308x over previous
"""Trainium2 Bass kernel for a group-conv / orbit-shared message-passing layer.

Math: out[b, i, o] = sum_{j,c} weight[o, c, pair_orbit[i, j]] * x[b, j, c] + bias[o]

Strategy (pure data parallel over 8 NeuronCores):
  * Host gathers the orbit-shared weight into per-output-position matrices
    W_i[(j,c), o] (24 matrices of 1536x64), regrouped as moving operands
    Wmov[k, g][kc, (di,o)] of [128, 512] covering 8 output positions each.
  * Host pre-tiles x so each batch tile is one fully-contiguous DMA:
    xtile[bt][kc, (k,b)] with the contraction dim (j,c)=1536 split into
    12 k-chunks of 128 on SBUF partitions; each core takes B/8 = 4096 rows.
  * Per 128-batch tile: stationary = x k-tile [kc=128, b=128], moving =
    Wmov[k, g] [kc=128, 512]; 12 k-tiles accumulate into 3 PSUM banks:
        psum_g[b, (di,o)] += x[kc, b].T @ Wmov[k,g][kc, (di,o)]
    The PSUM free axis (di,o) is already the natural out[b, i, o] layout, so
    stores go straight to a (4096, 24*64) DRAM tensor.
  * bf16 operands: full PE rate, FWL fast weight loads, half the HBM traffic
    of fp32. L2 rel err ~2.3e-3, well under the 2e-2 gate.
  * Startup: PE pre-warm dummy matmuls (p-state/HAM ramp), weight plane on
    the gpsimd DMA queue, and the first two batch tiles run k-interleaved so
    chunk consumption (6 matmuls/chunk) never outraces chunk delivery.
"""

import os
import sys

for _p in ("/opt/trn_rl_repo",):
    if _p not in sys.path:
        sys.path.insert(0, _p)

import numpy as np
import ml_dtypes

import concourse.bacc as bacc
import concourse.mybir as mybir
from concourse import tile
from concourse.bass_utils import run_bass_kernel_spmd

B, P, C_IN, C_OUT, N_ORB = 32768, 24, 64, 64, 24
N_CORES = 8
BL = B // N_CORES            # 4096 batch per core
JC = P * C_IN                # 1536 contraction size
KT = JC // 128               # 12 K-tiles
NG = 3                       # output groups of 8 positions (8*64 = 512 free)
NBT = BL // 128              # 32 batch tiles per core

# "bf16" | "f32r" | "f32"
COMPUTE_DTYPE = os.environ.get("COMPUTE_DTYPE", "bf16")
N_WARM = int(os.environ.get("N_WARM", "145"))

_CACHE = {}


def _dt(dt_tag):
    return {
        "bf16": mybir.dt.bfloat16,
        "f32r": mybir.dt.float32r,
        "f32": mybir.dt.float32,
    }[dt_tag]


def _build(dt_tag):
    DT = _dt(dt_tag)

    nc = bacc.Bacc(None, target_bir_lowering=False, debug=False)
    # pre-tiled x: row bt*128+kc, col k*128+b (one contiguous DMA per tile)
    xt = nc.dram_tensor("xt", [NBT * 128, KT * 128], DT, kind="ExternalInput")
    w = nc.dram_tensor("w", [128, KT * NG * 512], DT, kind="ExternalInput")
    out_l = nc.dram_tensor("out_l", [BL, P * C_OUT], mybir.dt.float32,
                           kind="ExternalOutput")

    with tile.TileContext(nc) as tc:
        with (
            tc.tile_pool(name="wpool", bufs=1) as wpool,
            tc.tile_pool(name="xpool", bufs=3) as xpool,
            tc.tile_pool(name="opool", bufs=6) as opool,
            tc.tile_pool(name="pspool", bufs=2, space="PSUM") as pspool,
        ):
            # all weight chunks on the dedicated gpsimd DMA queue; x/out
            # traffic rides the sync queue. (Do NOT move any weight chunk's
            # DMA onto the sync queue after the x splits — two attempts at
            # that produced a reproducible ~60us whole-run regression.)
            wk = []
            for k in range(KT):
                wc = wpool.tile([128, NG * 512], DT, tag=f"w{k}", name=f"wc{k}")
                nc.gpsimd.dma_start(
                    wc[:], w.ap()[:, k * NG * 512:(k + 1) * NG * 512]
                )
                wk.append(wc)

            def load_x(bt):
                xbt = xpool.tile([128, KT * 128], DT, tag="xbt", name=f"xb{bt}")
                nc.sync.dma_start(
                    xbt[:], xt.ap()[bt * 128:(bt + 1) * 128, :]
                )
                return xbt

            # x0/x1 split so their first k-slices land before the full tiles
            # (the PE hoists each LDWEIGHTS one matmul early, so tile bt+0's
            # last k0 matmul would otherwise block on ALL of x1)
            def load_x_split(bt):
                xbt = xpool.tile([128, KT * 128], DT, tag="xbt",
                                 name=f"xb{bt}")
                r = slice(bt * 128, (bt + 1) * 128)
                nc.sync.dma_start(xbt[:, 0:256], xt.ap()[r, 0:256])
                nc.sync.dma_start(xbt[:, 256:KT * 128],
                                  xt.ap()[r, 256:KT * 128])
                return xbt

            x0 = load_x_split(0)
            x1 = load_x_split(1)

            # PE pre-warm: dummy matmuls keep the PE busy from ~7us until the
            # first weight chunk lands (~13.5us) so the p-state/HAM ramp is
            # complete before the first real matmul issues (~44ns each)
            warm = xpool.tile([128, 64], DT, tag="warm", name="warm")
            nc.vector.memset(warm[:], 0.0)
            pswarm = pspool.tile([64, 64], mybir.dt.float32, tag="pswarm",
                                 name="pswarm")
            for _ in range(N_WARM):
                nc.tensor.matmul(pswarm[:], warm[:], warm[:],
                                 start=True, stop=True)

            def psum_tiles(bt):
                return [
                    pspool.tile([128, 512], mybir.dt.float32, tag=f"ps{g}",
                                name=f"ps{bt}_{g}")
                    for g in range(NG)
                ]

            def mm_k(ps, xtile, k):
                lhsT = xtile[:, k * 128:(k + 1) * 128]
                for g in range(NG):
                    nc.tensor.matmul(
                        ps[g][:],
                        lhsT,
                        wk[k][:, g * 512:(g + 1) * 512],
                        start=(k == 0),
                        stop=(k == KT - 1),
                    )

            def drain(bt, ps):
                for g in range(NG):
                    ob = opool.tile([128, 512], mybir.dt.float32, tag="ob",
                                    name=f"ob{bt}_{g}")
                    # drains split across DVE and ACT so consecutive drains
                    # overlap instead of serializing on one engine
                    if g == 1:
                        nc.scalar.copy(ob[:], ps[g][:])
                    else:
                        nc.vector.tensor_copy(ob[:], ps[g][:])
                    nc.sync.dma_start(
                        out_l.ap()[bt * 128:(bt + 1) * 128,
                                   g * 512:(g + 1) * 512],
                        ob[:],
                    )

            # first two batch tiles k-interleaved (see module docstring)
            ps0, ps1 = psum_tiles(0), psum_tiles(1)
            for k in range(KT):
                mm_k(ps0, x0, k)
                mm_k(ps1, x1, k)
            # x2/x3 ride the gpsimd queue BEHIND the weight chunks: they are
            # not needed until ~29/37us, and keeping their bytes out of the
            # early window lets w2 (the critical-path chunk) arrive sooner
            # on the shared early HBM bandwidth
            def load_x_late(bt):
                xbt = xpool.tile([128, KT * 128], DT, tag="xbt",
                                 name=f"xb{bt}")
                nc.gpsimd.dma_start(
                    xbt[:], xt.ap()[bt * 128:(bt + 1) * 128, :]
                )
                return xbt

            xtiles = {2: load_x_late(2)}
            drain(0, ps0)
            xtiles[3] = load_x_late(3)
            drain(1, ps1)

            for bt in range(2, NBT):
                xcur = xtiles.pop(bt)
                ps = psum_tiles(bt)
                for k in range(KT):
                    mm_k(ps, xcur, k)
                if bt + 2 < NBT:
                    xtiles[bt + 2] = load_x(bt + 2)
                drain(bt, ps)

    nc.compile()
    return nc


def _get_nc(dt_tag):
    if dt_tag not in _CACHE:
        _CACHE[dt_tag] = _build(dt_tag)
    return _CACHE[dt_tag]


def _pack_weight(weight, pair_orbit, dt_tag):
    # W_i[(j,c), o] = weight[o, c, pair_orbit[i, j]]
    kern = weight[:, :, np.asarray(pair_orbit)]          # (o, c, i, j)
    wfull = kern.transpose(2, 3, 1, 0).reshape(P, JC, C_OUT)   # (i, jc, o)
    # Wmov[k, g, kc, di*64+o] = wfull[g*8+di, k*128+kc, o]
    wmov = (
        wfull.reshape(NG, 8, KT, 128, C_OUT)
        .transpose(2, 0, 3, 1, 4)
        .reshape(KT * NG, 128, 512)
    )
    wsb = np.ascontiguousarray(
        wmov.transpose(1, 0, 2).reshape(128, KT * NG * 512), dtype=np.float32
    )
    if dt_tag == "bf16":
        return wsb.astype(ml_dtypes.bfloat16)
    return wsb


def _shard_x(x, dt_tag):
    # xtile[bt*128+kc, k*128+b] = x[core*BL + bt*128 + b, k*128 + kc]
    x2 = x.reshape(B, JC)
    if dt_tag == "bf16":
        x2 = x2.astype(ml_dtypes.bfloat16)
    shards = []
    for c in range(N_CORES):
        xc = x2[c * BL:(c + 1) * BL]
        xtile = (
            xc.reshape(NBT, 128, KT, 128)       # (bt, b, k, kc)
            .transpose(0, 3, 2, 1)              # (bt, kc, k, b)
            .reshape(NBT * 128, KT * 128)
        )
        shards.append(np.ascontiguousarray(xtile))
    return shards


def _prepare_inputs(x, weight, pair_orbit, dt_tag):
    wsb = _pack_weight(weight, pair_orbit, dt_tag)
    xts = _shard_x(x, dt_tag)
    return [{"xt": xts[c], "w": wsb} for c in range(N_CORES)]


def kernel(x, weight, bias, pair_orbit):
    x = np.asarray(x, dtype=np.float32)
    weight = np.asarray(weight, dtype=np.float32)
    bias = np.asarray(bias, dtype=np.float32)

    dt_tag = COMPUTE_DTYPE
    nc = _get_nc(dt_tag)
    in_maps = _prepare_inputs(x, weight, pair_orbit, dt_tag)

    res = run_bass_kernel_spmd(nc, in_maps, core_ids=list(range(N_CORES)))

    out = np.concatenate(
        [res.results[c]["out_l"] for c in range(N_CORES)], axis=0
    ).reshape(B, P, C_OUT)
    if bias.any():
        out = out + bias
    return out
